# revision 4
# baseline (speedup 1.0000x reference)
"""NextVLAD Trainium2 kernel — 8-way data-parallel over batch (1 sample/core).

Per-core dataflow (sample b, M=512 tokens, N=1024 feat, E*N=2048, G=8, K=128, D=256):
  X [n,m]      <- host-reordered input slice (n on partitions, tokens m on free)
  inv[*,m]     = rsqrt(sum_n X^2)    via ones-matmul partition reduction (broadcast rows)
  y[e,m]       = (W_inp.T-chunks)^T X * inv            (no bias: b_inp cancels in softmax)
  yT[m,e]      = X-slices^T W_inp.T * inv + b_inp      (einsum operand, interleaved with
                                                        a ones column per group block)
  sg[g,m]      = sigmoid((W_g@W_inp) X * inv + b_g')   b_g' = b_g + W_g@b_inp (host)
  logits[m,gk] = y^T W_gk.T-chunks ; ex = exp(logits)  (softmax max-sub skipped; b_gk cancels)
  ise          = 1/sum_m ex          via ones-matmul
  wf[m,gk]     = ex * sg * ise
  vlad[k,d+1]  = sum_{g,m} wf^T [yT | 1]   (ones col gives S[k] = sum w in col D)
  out[k,d]     = l2norm_d(vlad - S*cent) / sqrt(128)   (global l2norm == /sqrt(128))

Big matmuls run as float32r (full PE rate at free-dim>=256).
"""
import os
import numpy as np

N = 1024          # feature size
EN = 2048         # expanded features
G = 8             # groups
KC = 128          # clusters
D = 256           # per-group cluster dim
M = 512           # tokens per sample (8*8*8)
NT = N // 128     # 8 contraction chunks over n
ET = EN // 128    # 16 e-tiles
EC = 4            # e-chunks of 512
MT = 4            # m-tiles of 128
GH = 2            # gk halves of 512

_cache = {}


def _build_nc():
    import concourse.bacc as bacc
    import concourse.tile as tile
    from concourse import mybir

    f32 = mybir.dt.float32
    f32r = mybir.dt.float32r
    Alu = mybir.AluOpType
    Act = mybir.ActivationFunctionType

    nc = bacc.Bacc("TRN2", target_bir_lowering=False)
    x_d = nc.dram_tensor("x", [N, M], f32r, kind="ExternalInput")
    w1_d = nc.dram_tensor("w1", [N, EN], f32r, kind="ExternalInput")
    w2_d = nc.dram_tensor("w2", [EN, G * KC], f32r, kind="ExternalInput")
    wg_d = nc.dram_tensor("wg", [N, G], f32r, kind="ExternalInput")
    binp_d = nc.dram_tensor("binp", [1, EN], f32, kind="ExternalInput")
    bg_d = nc.dram_tensor("bg", [G, 1], f32, kind="ExternalInput")
    centn_d = nc.dram_tensor("centn", [KC, D], f32, kind="ExternalInput")
    ident_d = nc.dram_tensor("ident", [128, 128], f32, kind="ExternalInput")
    ones_d = nc.dram_tensor("ones", [128, 128], f32r, kind="ExternalInput")
    out_d = nc.dram_tensor("out", [KC, D], f32, kind="ExternalOutput")

    with tile.TileContext(nc) as tc:
        with tc.tile_pool(name="const", bufs=1) as constp, \
             tc.tile_pool(name="persist", bufs=1) as persist, \
             tc.tile_pool(name="w2p", bufs=4) as w2p, \
             tc.tile_pool(name="fin", bufs=1) as fin:
            binp_b = constp.tile([128, EN], f32)
            nc.gpsimd.dma_start(out=binp_b[:], in_=binp_d[:].to_broadcast([128, EN]))
            bg_t = constp.tile([G, 1], f32)
            nc.sync.dma_start(out=bg_t[:], in_=bg_d[:])
            centn_t = constp.tile([KC, D], f32)
            nc.sync.dma_start(out=centn_t[:], in_=centn_d[:])
            ident_t = constp.tile([128, 128], f32)
            nc.sync.dma_start(out=ident_t[:], in_=ident_d[:])
            ones_t = constp.tile([128, 128], f32r)
            nc.sync.dma_start(out=ones_t[:], in_=ones_d[:])

            y_t = [persist.tile([128, M], f32r, name=f"y{e}") for e in range(ET)]
            yT_t = [persist.tile([128, G * (D + 2)], f32r, name=f"yT{m}") for m in range(MT)]
            sgc_t = [persist.tile([128, G], f32, name=f"sgc{m}") for m in range(MT)]
            inv_t = persist.tile([128, M], f32)
            invc_t = [persist.tile([128, 1], f32, name=f"invc{m}") for m in range(MT)]

            # ---------------- phase 1: input, norm, gates, fc_inp ----------------
            with tc.tile_pool(name="xw", bufs=1) as xw, \
                 tc.tile_pool(name="sm1", bufs=1) as sm1, \
                 tc.tile_pool(name="ps1", bufs=3, space="PSUM") as ps1, \
                 tc.tile_pool(name="ps1s", bufs=1, space="PSUM") as ps1s:
                x_t = [xw.tile([128, M], f32r, name=f"x{i}") for i in range(NT)]
                for i in range(NT):
                    nc.sync.dma_start(out=x_t[i][:], in_=x_d[i * 128:(i + 1) * 128, :])
                w1_t = [xw.tile([128, EN], f32r, name=f"w1_{i}") for i in range(NT)]
                for i in range(NT):
                    nc.sync.dma_start(out=w1_t[i][:], in_=w1_d[i * 128:(i + 1) * 128, :])
                wg_t = [xw.tile([128, G], f32r, name=f"wg{i}") for i in range(NT)]
                for i in range(NT):
                    nc.sync.dma_start(out=wg_t[i][:], in_=wg_d[i * 128:(i + 1) * 128, :])

                # sum of squares over n (partition reduction via ones-matmul)
                ss_ps = ps1.tile([128, M], f32, name="ss_ps", bufs=1)
                for i in range(NT):
                    xsq = sm1.tile([128, M], f32r, name="xsq", bufs=2)
                    nc.scalar.activation(xsq[:], x_t[i][:], Act.Square)
                    nc.tensor.matmul(ss_ps[:], ones_t[:], xsq[:],
                                     start=(i == 0), stop=(i == NT - 1))
                nrm_t = sm1.tile([128, M], f32, name="nrm", bufs=1)
                nc.scalar.activation(nrm_t[:], ss_ps[:], Act.Sqrt)
                nc.vector.reciprocal(inv_t[:], nrm_t[:])

                # per-partition columns of inv (transpose rows via K=1 matmul)
                for m in range(MT):
                    ic_ps = ps1s.tile([128, 2], f32, name="ic_ps", bufs=1)
                    nc.tensor.matmul(ic_ps[:], inv_t[0:1, m * 128:(m + 1) * 128],
                                     ident_t[0:1, 0:2], start=True, stop=True)
                    nc.vector.tensor_copy(invc_t[m][:], ic_ps[:, 0:1])

                # sigmoid gate logits: [G, M] = (W_g@W_inp) X, then *inv, sigmoid(+bias)
                sg_ps = ps1s.tile([G, M], f32, name="sg_ps", bufs=1)
                for i in range(NT):
                    nc.tensor.matmul(sg_ps[:], wg_t[i][:], x_t[i][:],
                                     start=(i == 0), stop=(i == NT - 1))
                sgs_t = sm1.tile([G, M], f32, name="sgs", bufs=1)
                nc.vector.tensor_mul(sgs_t[:], sg_ps[:], inv_t[0:G, :])
                nc.scalar.activation(sgs_t[:], sgs_t[:], Act.Sigmoid, bias=bg_t[:])
                # transpose [G, M] -> per m-tile [128, G]
                for m in range(MT):
                    sgc_ps = ps1s.tile([128, G], f32, name="sgc_ps", bufs=1)
                    nc.tensor.matmul(sgc_ps[:], sgs_t[:, m * 128:(m + 1) * 128],
                                     ident_t[0:G, 0:G], start=True, stop=True)
                    nc.vector.tensor_copy(sgc_t[m][:], sgc_ps[:])

                # y[e,m] chains (for logits): no bias, scaled by inv
                for e in range(ET):
                    y_ps = ps1.tile([128, M], f32, name="mm_ps", bufs=3)
                    for i in range(NT):
                        nc.tensor.matmul(y_ps[:], w1_t[i][:, e * 128:(e + 1) * 128],
                                         x_t[i][:], start=(i == 0), stop=(i == NT - 1))
                    nc.vector.tensor_mul(y_t[e][:], y_ps[:], inv_t[:])

                # yT[m,e] chains (einsum operand): bias + inv, group-interleaved layout
                for m in range(MT):
                    yT3 = yT_t[m].rearrange("p (g c) -> p g c", c=D + 2)
                    nc.vector.tensor_copy(yT3[:, :, D:D + 2],
                                          ones_t[:, 0:2 * G].rearrange("p (g c) -> p g c", c=2))
                    for c in range(EC):
                        t_ps = ps1.tile([128, 512], f32, name="mm_ps", bufs=3)
                        for i in range(NT):
                            nc.tensor.matmul(t_ps[:], x_t[i][:, m * 128:(m + 1) * 128],
                                             w1_t[i][:, c * 512:(c + 1) * 512],
                                             start=(i == 0), stop=(i == NT - 1))
                        dst = yT3[:, 2 * c:2 * c + 2, 0:D]
                        src_b = binp_b[:, c * 512:(c + 1) * 512].rearrange(
                            "p (g c2) -> p g c2", c2=D)
                        ps_v = t_ps[:].rearrange("p (g c2) -> p g c2", c2=D)
                        nc.vector.scalar_tensor_tensor(
                            out=dst, in0=ps_v, scalar=invc_t[m][:], in1=src_b,
                            op0=Alu.mult, op1=Alu.add)

            # ---------------- phase 2: gk logits + exp ----------------
            with tc.tile_pool(name="exp2", bufs=1) as exp2:
                ex_t = [exp2.tile([128, G * KC], f32r, name=f"ex{m}") for m in range(MT)]
                with tc.tile_pool(name="ps2", bufs=1, space="PSUM") as ps2:
                    lg_ps = [[ps2.tile([128, 512], f32, name=f"lg{m}_{h}", bufs=1)
                              for h in range(GH)] for m in range(MT)]
                    for e in range(ET):
                        w2t = w2p.tile([128, G * KC], f32r, name="w2t")
                        nc.sync.dma_start(out=w2t[:], in_=w2_d[e * 128:(e + 1) * 128, :])
                        for m in range(MT):
                            for h in range(GH):
                                nc.tensor.matmul(
                                    lg_ps[m][h][:], y_t[e][:, m * 128:(m + 1) * 128],
                                    w2t[:, h * 512:(h + 1) * 512],
                                    start=(e == 0), stop=(e == ET - 1))
                    for m in range(MT):
                        for h in range(GH):
                            nc.scalar.activation(ex_t[m][:, h * 512:(h + 1) * 512],
                                                 lg_ps[m][h][:], Act.Exp)

                # ---------------- phase 3: softmax denom, weights, einsum ----------------
                with tc.tile_pool(name="p3", bufs=1) as p3, \
                     tc.tile_pool(name="ps3", bufs=1, space="PSUM") as ps3:
                    ise_t = p3.tile([128, G * KC], f32)
                    for h in range(GH):
                        se_ps = ps3.tile([128, 512], f32, name="se_ps", bufs=2)
                        for m in range(MT):
                            nc.tensor.matmul(se_ps[:], ones_t[:],
                                             ex_t[m][:, h * 512:(h + 1) * 512],
                                             start=(m == 0), stop=(m == MT - 1))
                        nc.vector.reciprocal(ise_t[:, h * 512:(h + 1) * 512], se_ps[:])

                    wf_t = [p3.tile([128, G * KC], f32r, name=f"wf{m}") for m in range(MT)]
                    for m in range(MT):
                        for g in range(G):
                            sl = slice(g * KC, (g + 1) * KC)
                            nc.vector.scalar_tensor_tensor(
                                out=wf_t[m][:, sl], in0=ex_t[m][:, sl],
                                scalar=sgc_t[m][:, g:g + 1], in1=ise_t[:, sl],
                                op0=Alu.mult, op1=Alu.mult)

                    vd_ps = ps3.tile([128, D + 2], f32, name="vd_ps", bufs=1)
                    k = 0
                    for g in range(G):
                        for m in range(MT):
                            nc.tensor.matmul(
                                vd_ps[:], wf_t[m][:, g * KC:(g + 1) * KC],
                                yT_t[m][:, g * (D + 2):(g + 1) * (D + 2)],
                                start=(k == 0), stop=(k == G * MT - 1))
                            k += 1

                    vlad_t = fin.tile([128, D], f32)
                    nc.vector.scalar_tensor_tensor(
                        out=vlad_t[:], in0=centn_t[:], scalar=vd_ps[:, D:D + 1],
                        in1=vd_ps[:, 0:D], op0=Alu.mult, op1=Alu.add)
                    sq_t = fin.tile([128, D], f32)
                    nc.vector.tensor_mul(sq_t[:], vlad_t[:], vlad_t[:])
                    ss2_t = fin.tile([128, 1], f32)
                    nc.vector.reduce_sum(out=ss2_t[:], in_=sq_t[:],
                                         axis=mybir.AxisListType.X)
                    nr2_t = fin.tile([128, 1], f32)
                    nc.scalar.activation(nr2_t[:], ss2_t[:], Act.Sqrt, scale=128.0)
                    r1_t = fin.tile([128, 1], f32)
                    nc.vector.reciprocal(r1_t[:], nr2_t[:])
                    out_t = fin.tile([128, D], f32)
                    nc.vector.tensor_scalar_mul(out_t[:], vlad_t[:], r1_t[:])
                    nc.sync.dma_start(out=out_d[:], in_=out_t[:])

    nc.compile()
    return nc


def _get_nc():
    if "nc" not in _cache:
        _cache["nc"] = _build_nc()
    return _cache["nc"]


def kernel(x, W_inp, b_inp, W_g, b_g, W_gk, b_gk, centroids):
    from concourse.bass_utils import run_bass_kernel_spmd

    nc = _get_nc()

    x = np.asarray(x, dtype=np.float32)
    X = x.reshape(8, 8, N, 64).transpose(0, 2, 1, 3).reshape(8, N, M)
    W1 = np.ascontiguousarray(np.asarray(W_inp, np.float32).T)
    W2 = np.ascontiguousarray(np.asarray(W_gk, np.float32).T)
    WgT = np.ascontiguousarray(
        (np.asarray(W_g, np.float64) @ np.asarray(W_inp, np.float64)).T
    ).astype(np.float32)
    bg = (np.asarray(b_g, np.float64)
          + np.asarray(W_g, np.float64) @ np.asarray(b_inp, np.float64)
          ).astype(np.float32).reshape(G, 1)
    binp = np.ascontiguousarray(np.asarray(b_inp, np.float32).reshape(1, EN))
    centn = np.ascontiguousarray(-np.asarray(centroids, np.float32))
    ident = np.eye(128, dtype=np.float32)
    ones = np.ones((128, 128), dtype=np.float32)

    in_maps = []
    for b in range(8):
        in_maps.append({
            "x": np.ascontiguousarray(X[b]), "w1": W1, "w2": W2, "wg": WgT,
            "binp": binp, "bg": bg, "centn": centn, "ident": ident, "ones": ones,
        })

    trace = os.environ.get("KERNEL_TRACE") == "1"
    r = run_bass_kernel_spmd(nc, in_maps, core_ids=list(range(8)), trace=trace)
    _cache["last_results"] = r
    return np.stack([r.results[b]["out"].reshape(KC * D) for b in range(8)]).astype(np.float32)


# revision 6
# speedup vs baseline: 1.1946x; 1.1946x over previous
"""NextVLAD Trainium2 kernel — 8-way data-parallel over batch (1 sample/core).

Per-core dataflow (sample b, M=512 tokens, N=1024 feat, E*N=2048, G=8, K=128, D=256):
  X [n,m]      <- host-reordered input slice (n on partitions, tokens m on free)
  inv[*,m]     = 1/sqrt(sum_n X^2)   via ones-matmul partition reduction (broadcast rows)
  y[e,m]       = (W_inp.T-chunks)^T X * inv            (no bias: b_inp cancels in softmax)
  yT[m,e]      = PE-transpose(y) + b_inp               (einsum operand, group-interleaved
                                                        layout with ones columns)
  sg[g,m]      = sigmoid((W_g@W_inp) X * inv + b_g')   b_g' = b_g + W_g@b_inp (host)
  logits[m,gk] = y^T W_gk.T-chunks ; ex = exp(logits)  (softmax max-sub skipped; b_gk cancels)
  ise          = 1/sum_m ex          via ones-matmul
  wf[m,gk]     = ex * sg * ise
  vlad[k,d+2]  = sum_{g,m} wf^T [yT | 1 1]  (ones cols give S[k] = sum w in col D)
  out[k,d]     = l2norm_d(vlad - S*cent) / sqrt(128)   (global l2norm == /sqrt(128))

Big matmuls run as float32r (full PE rate at even free-dim>=256).
"""
import os
import numpy as np

N = 1024          # feature size
EN = 2048         # expanded features
G = 8             # groups
KC = 128          # clusters
D = 256           # per-group cluster dim
BW = D + 2        # group block width in yT (data + ones + pad)
M = 512           # tokens per sample (8*8*8)
NT = N // 128     # 8 contraction chunks over n
ET = EN // 128    # 16 e-tiles
MT = 4            # m-tiles of 128
GH = 2            # gk halves of 512

_cache = {}


def _build_nc():
    import concourse.bacc as bacc
    import concourse.tile as tile
    from concourse import mybir

    f32 = mybir.dt.float32
    f32r = mybir.dt.float32r
    Alu = mybir.AluOpType
    Act = mybir.ActivationFunctionType

    nc = bacc.Bacc("TRN2", target_bir_lowering=False)
    x_d = nc.dram_tensor("x", [N, M], f32r, kind="ExternalInput")
    w1_d = nc.dram_tensor("w1", [N, EN], f32r, kind="ExternalInput")
    w2_d = nc.dram_tensor("w2", [EN, G * KC], f32r, kind="ExternalInput")
    wg_d = nc.dram_tensor("wg", [N, G], f32r, kind="ExternalInput")
    binp_d = nc.dram_tensor("binp", [1, EN], f32, kind="ExternalInput")
    bg_d = nc.dram_tensor("bg", [G, 1], f32, kind="ExternalInput")
    centn_d = nc.dram_tensor("centn", [KC, D], f32, kind="ExternalInput")
    identf_d = nc.dram_tensor("identf", [128, 128], f32, kind="ExternalInput")
    identr_d = nc.dram_tensor("identr", [128, 128], f32r, kind="ExternalInput")
    ones_d = nc.dram_tensor("ones", [128, 128], f32r, kind="ExternalInput")
    out_d = nc.dram_tensor("out", [KC, D], f32, kind="ExternalOutput")

    with tile.TileContext(nc) as tc:
        with tc.tile_pool(name="const", bufs=1) as constp, \
             tc.tile_pool(name="persist", bufs=1) as persist, \
             tc.tile_pool(name="w2p", bufs=3) as w2p, \
             tc.tile_pool(name="fin", bufs=1) as fin:
            # small consts; ones first (gates the first PE op)
            ones_t = constp.tile([128, 128], f32r)
            nc.sync.dma_start(out=ones_t[:], in_=ones_d[:])
            identf_t = constp.tile([128, 128], f32)
            nc.sync.dma_start(out=identf_t[:], in_=identf_d[:])
            identr_t = constp.tile([128, 128], f32r)
            nc.sync.dma_start(out=identr_t[:], in_=identr_d[:])
            bg_t = constp.tile([G, 1], f32)
            nc.sync.dma_start(out=bg_t[:], in_=bg_d[:])
            centn_t = constp.tile([KC, D], f32)
            nc.sync.dma_start(out=centn_t[:], in_=centn_d[:])
            binp_b = constp.tile([128, EN], f32)
            nc.gpsimd.dma_start(out=binp_b[:], in_=binp_d[:].to_broadcast([128, EN]))

            y_t = [persist.tile([128, M], f32r, name=f"y{e}") for e in range(ET)]
            yT_t = [persist.tile([128, G * BW], f32r, name=f"yT{m}") for m in range(MT)]
            sgc_t = [persist.tile([128, G], f32, name=f"sgc{m}") for m in range(MT)]
            inv_t = persist.tile([128, M], f32)

            # ---------------- phase 1: input, norm, fc_inp, gates, yT ----------------
            with tc.tile_pool(name="xw", bufs=1) as xw, \
                 tc.tile_pool(name="sm1", bufs=1) as sm1, \
                 tc.tile_pool(name="ps1", bufs=1, space="PSUM") as ps1:
                # DMA order: x tiles, then w1 first-halves, then second-halves
                x_t = [xw.tile([128, M], f32r, name=f"x{i}") for i in range(NT)]
                for i in range(NT):
                    nc.sync.dma_start(out=x_t[i][:], in_=x_d[i * 128:(i + 1) * 128, :])
                w1_t = [xw.tile([128, EN], f32r, name=f"w1_{i}") for i in range(NT)]
                for i in range(NT):
                    nc.sync.dma_start(out=w1_t[i][:, 0:1024],
                                      in_=w1_d[i * 128:(i + 1) * 128, 0:1024])
                for i in range(NT):
                    nc.sync.dma_start(out=w1_t[i][:, 1024:2048],
                                      in_=w1_d[i * 128:(i + 1) * 128, 1024:2048])
                wg_t = [xw.tile([128, G], f32r, name=f"wg{i}") for i in range(NT)]
                for i in range(NT):
                    nc.sync.dma_start(out=wg_t[i][:], in_=wg_d[i * 128:(i + 1) * 128, :])

                # sum of squares over n (partition reduction via ones-matmul)
                ss_ps = ps1.tile([128, M], f32, name="mm_ps", tag="mm_ps", bufs=8)
                for i in range(NT):
                    xsq = sm1.tile([128, M], f32r, name="xsq", bufs=2)
                    nc.scalar.activation(xsq[:], x_t[i][:], Act.Square)
                    nc.tensor.matmul(ss_ps[:], ones_t[:], xsq[:],
                                     start=(i == 0), stop=(i == NT - 1))
                nrm_t = sm1.tile([128, M], f32, name="nrm", bufs=1)
                nc.scalar.activation(nrm_t[:], ss_ps[:], Act.Sqrt)
                nc.vector.reciprocal(inv_t[:], nrm_t[:])

                # y[e,m] chains, two sweeps of 8, i-outer to match w1 DMA arrival
                for lo in (0, 8):
                    y_ps = [ps1.tile([128, M], f32, name=f"y_ps{e}", tag="mm_ps", bufs=8)
                            for e in range(lo, lo + 8)]
                    for i in range(NT):
                        for k, e in enumerate(range(lo, lo + 8)):
                            nc.tensor.matmul(y_ps[k][:],
                                             w1_t[i][:, e * 128:(e + 1) * 128],
                                             x_t[i][:], start=(i == 0),
                                             stop=(i == NT - 1))
                    for k, e in enumerate(range(lo, lo + 8)):
                        nc.vector.tensor_mul(y_t[e][:], y_ps[k][:], inv_t[:])

                # sigmoid gate logits: [G, M] = (W_g@W_inp) X, *inv, sigmoid(+bias)
                sg_ps = ps1.tile([G, M], f32, name="sg_ps", tag="mm_ps", bufs=8)
                for i in range(NT):
                    nc.tensor.matmul(sg_ps[:], wg_t[i][:], x_t[i][:],
                                     start=(i == 0), stop=(i == NT - 1))
                sgs_t = sm1.tile([G, M], f32, name="sgs", bufs=1)
                nc.vector.tensor_mul(sgs_t[:], sg_ps[:], inv_t[0:G, :])
                nc.scalar.activation(sgs_t[:], sgs_t[:], Act.Sigmoid, bias=bg_t[:])
                # transpose [G, M] -> per m-tile [128, G]
                for m in range(MT):
                    sgc_ps = ps1.tile([128, G], f32, name="sgc_ps", tag="mm_ps", bufs=8)
                    nc.tensor.matmul(sgc_ps[:], sgs_t[:, m * 128:(m + 1) * 128],
                                     identf_t[0:G, 0:G], start=True, stop=True)
                    nc.vector.tensor_copy(sgc_t[m][:], sgc_ps[:])

                # yT via PE transposes of y, bias added on eviction
                for m in range(MT):
                    yT3 = yT_t[m].rearrange("p (g c) -> p g c", c=BW)
                    nc.vector.tensor_copy(yT3[:, :, D:D + 2],
                                          ones_t[:, 0:2 * G].rearrange(
                                              "p (g c) -> p g c", c=2))
                for et in range(ET):
                    g, half = et // 2, et % 2
                    for m in range(MT):
                        t_ps = ps1.tile([128, 128], f32r, name="t_ps", tag="mm_ps", bufs=8)
                        nc.tensor.transpose(t_ps[:], y_t[et][:, m * 128:(m + 1) * 128],
                                            identr_t[:])
                        col = g * BW + half * 128
                        nc.vector.tensor_add(yT_t[m][:, col:col + 128], t_ps[:],
                                             binp_b[:, et * 128:(et + 1) * 128])

            # ---------------- phase 2: gk logits + exp ----------------
            with tc.tile_pool(name="exp2", bufs=1) as exp2:
                ex_t = [exp2.tile([128, G * KC], f32r, name=f"ex{m}") for m in range(MT)]
                with tc.tile_pool(name="ps2", bufs=1, space="PSUM") as ps2:
                    lg_ps = [[ps2.tile([128, 512], f32, name=f"lg{m}_{h}", bufs=1)
                              for h in range(GH)] for m in range(MT)]
                    for e in range(ET):
                        w2t = w2p.tile([128, G * KC], f32r, name="w2t")
                        nc.sync.dma_start(out=w2t[:], in_=w2_d[e * 128:(e + 1) * 128, :])
                        for m in range(MT):
                            for h in range(GH):
                                nc.tensor.matmul(
                                    lg_ps[m][h][:], y_t[e][:, m * 128:(m + 1) * 128],
                                    w2t[:, h * 512:(h + 1) * 512],
                                    start=(e == 0), stop=(e == ET - 1))
                    for m in range(MT):
                        for h in range(GH):
                            nc.scalar.activation(ex_t[m][:, h * 512:(h + 1) * 512],
                                                 lg_ps[m][h][:], Act.Exp)

                # ---------------- phase 3: softmax denom, weights, einsum ----------------
                with tc.tile_pool(name="p3", bufs=1) as p3, \
                     tc.tile_pool(name="ps3", bufs=1, space="PSUM") as ps3:
                    ise_t = p3.tile([128, G * KC], f32)
                    for h in range(GH):
                        se_ps = ps3.tile([128, 512], f32, name="se_ps", bufs=2)
                        for m in range(MT):
                            nc.tensor.matmul(se_ps[:], ones_t[:],
                                             ex_t[m][:, h * 512:(h + 1) * 512],
                                             start=(m == 0), stop=(m == MT - 1))
                        nc.vector.reciprocal(ise_t[:, h * 512:(h + 1) * 512], se_ps[:])

                    wf_t = [p3.tile([128, G * KC], f32r, name=f"wf{m}") for m in range(MT)]
                    for m in range(MT):
                        for g in range(G):
                            sl = slice(g * KC, (g + 1) * KC)
                            nc.vector.scalar_tensor_tensor(
                                out=wf_t[m][:, sl], in0=ex_t[m][:, sl],
                                scalar=sgc_t[m][:, g:g + 1], in1=ise_t[:, sl],
                                op0=Alu.mult, op1=Alu.mult)

                    vd_ps = ps3.tile([128, BW], f32, name="vd_ps", bufs=1)
                    k = 0
                    for g in range(G):
                        for m in range(MT):
                            nc.tensor.matmul(
                                vd_ps[:], wf_t[m][:, g * KC:(g + 1) * KC],
                                yT_t[m][:, g * BW:(g + 1) * BW],
                                start=(k == 0), stop=(k == G * MT - 1))
                            k += 1

                    vlad_t = fin.tile([128, D], f32)
                    nc.vector.scalar_tensor_tensor(
                        out=vlad_t[:], in0=centn_t[:], scalar=vd_ps[:, D:D + 1],
                        in1=vd_ps[:, 0:D], op0=Alu.mult, op1=Alu.add)
                    sq_t = fin.tile([128, D], f32)
                    nc.vector.tensor_mul(sq_t[:], vlad_t[:], vlad_t[:])
                    ss2_t = fin.tile([128, 1], f32)
                    nc.vector.reduce_sum(out=ss2_t[:], in_=sq_t[:],
                                         axis=mybir.AxisListType.X)
                    nr2_t = fin.tile([128, 1], f32)
                    nc.scalar.activation(nr2_t[:], ss2_t[:], Act.Sqrt, scale=128.0)
                    r1_t = fin.tile([128, 1], f32)
                    nc.vector.reciprocal(r1_t[:], nr2_t[:])
                    out_t = fin.tile([128, D], f32)
                    nc.vector.tensor_scalar_mul(out_t[:], vlad_t[:], r1_t[:])
                    nc.sync.dma_start(out=out_d[:], in_=out_t[:])

    nc.compile()
    return nc


def _get_nc():
    if "nc" not in _cache:
        _cache["nc"] = _build_nc()
    return _cache["nc"]


def kernel(x, W_inp, b_inp, W_g, b_g, W_gk, b_gk, centroids):
    from concourse.bass_utils import run_bass_kernel_spmd

    nc = _get_nc()

    x = np.asarray(x, dtype=np.float32)
    X = x.reshape(8, 8, N, 64).transpose(0, 2, 1, 3).reshape(8, N, M)
    W1 = np.ascontiguousarray(np.asarray(W_inp, np.float32).T)
    W2 = np.ascontiguousarray(np.asarray(W_gk, np.float32).T)
    WgT = np.ascontiguousarray(
        (np.asarray(W_g, np.float64) @ np.asarray(W_inp, np.float64)).T
    ).astype(np.float32)
    bg = (np.asarray(b_g, np.float64)
          + np.asarray(W_g, np.float64) @ np.asarray(b_inp, np.float64)
          ).astype(np.float32).reshape(G, 1)
    binp = np.ascontiguousarray(np.asarray(b_inp, np.float32).reshape(1, EN))
    centn = np.ascontiguousarray(-np.asarray(centroids, np.float32))
    ident = np.eye(128, dtype=np.float32)
    ones = np.ones((128, 128), dtype=np.float32)

    in_maps = []
    for b in range(8):
        in_maps.append({
            "x": np.ascontiguousarray(X[b]), "w1": W1, "w2": W2, "wg": WgT,
            "binp": binp, "bg": bg, "centn": centn,
            "identf": ident, "identr": ident, "ones": ones,
        })

    trace = os.environ.get("KERNEL_TRACE") == "1"
    r = run_bass_kernel_spmd(nc, in_maps, core_ids=list(range(8)), trace=trace)
    _cache["last_results"] = r
    return np.stack([r.results[b]["out"].reshape(KC * D) for b in range(8)]).astype(np.float32)
